# revision 5
# baseline (speedup 1.0000x reference)
"""Trainium2 Bass kernel for nn_AttentionBlock (GroupNorm -> 1x1 qkv conv ->
softmax attention over N=HW -> 1x1 proj -> residual).

Sharding: 8 cores = 4 images x 2 query-column halves. Each core receives its
image column-permuted so its own 2048 query columns come first; attention is
permutation-invariant over key/value positions, so k/v use all 4096 columns
in permuted order.

v3 design (fp8 DoubleRow attention, pipelined generation):
  - x uploaded twice: bf16 (stats + residual) and fp8e4m3 (matmul operand).
  - GroupNorm folded into qkv weights at runtime (wqk_s = wqk * r_c); q/k
    carry a x4 fp8 scale compensated in the exp scale 1/256. The -mu*r
    correction folds into the q bias (matvec) and proj bias (matvec chain);
    for k it is softmax-invariant and dropped.
  - q/k/v fp8 in [128, 2, n] channel-pair layout; qk/av/S matmuls run fp8
    DoubleRow (contraction 256/pass). S accumulates on the PE via an
    all-ones fp8 matmul into a replicated [128,512] PSUM bank.
  - exp on ACT per PAIR of key chunks ([128,2,512] PSUM read); ACT does
    nothing else in steady state.
  - k/q/v generation is interleaved INTO attention tile 0's pair loop
    (v-gen on the qk PSUM tag, k/q halves on the pp tag) so the exp stream
    starts ~20us in and generation copies hide under it.
  - av/S accumulation lags exp by 4 pairs so the tile-boundary recip+mul
    (DVE) never stalls the PE's write-after-read on the accumulators.
  - proj stays bf16; y stored bf16, upcast on host.
"""

import numpy as np

B, C, HH, WW = 4, 256, 64, 64
N = HH * WW            # 4096
NH = N // 2            # 2048 query columns per core
GROUPS = 32
GSIZE = C // GROUPS    # 8
EPS = 1e-5
NCORES = 8
P = 128
NT = NH // 512         # 4 query tiles per core
MC = N // P            # 32 key chunks
MCP = MC // 2          # 16 key-chunk pairs
KT = N // 512          # 8 column tiles for k
SQ = 4.0               # q,k fp8 pre-scale (folded into wqk host upload)
MUS = 32.0             # mu fp8 pre-scale (compensated in bias fixups)
LAG = 4                # av/S accumulation lag behind exp, in pairs

_prog = None


def _build_program():
    import concourse.bacc as bacc
    import concourse.tile as tile
    from concourse import mybir

    f32 = mybir.dt.float32
    bf16 = mybir.dt.bfloat16
    f8 = mybir.dt.float8e4
    AF = mybir.ActivationFunctionType
    ALU = mybir.AluOpType
    DR = mybir.MatmulPerfMode.DoubleRow

    nc = bacc.Bacc("TRN2", target_bir_lowering=False, debug=False,
                   num_devices=NCORES)

    xb_d = nc.dram_tensor("xb", [C, N], bf16, kind="ExternalInput").ap()
    x8_d = nc.dram_tensor("x8", [C, N], f8, kind="ExternalInput").ap()
    wqk_d = nc.dram_tensor("wqk", [C, 2 * C], bf16, kind="ExternalInput").ap()
    wv_d = nc.dram_tensor("wv", [C, C], bf16, kind="ExternalInput").ap()
    wp_d = nc.dram_tensor("wp", [C, C], bf16, kind="ExternalInput").ap()
    bq4_d = nc.dram_tensor("bq4", [C, 1], f32, kind="ExternalInput").ap()
    bp0_d = nc.dram_tensor("bp0", [C, 1], f32, kind="ExternalInput").ap()
    gm_d = nc.dram_tensor("gm", [P, 16], f32, kind="ExternalInput").ap()
    gt_d = nc.dram_tensor("gt", [16, P], f32, kind="ExternalInput").ap()
    y_d = nc.dram_tensor("y", [C, NH], bf16, kind="ExternalOutput").ap()

    xbv = xb_d.rearrange("(j p) n -> p j n", p=P)      # [128, 2, 4096]
    x8v = x8_d.rearrange("(j p) n -> p j n", p=P)
    wqkv = wqk_d.rearrange("(j p) o -> p j o", p=P)    # [128, 2, 512]
    wvv = wv_d.rearrange("(j p) o -> p j o", p=P)
    wpv = wp_d.rearrange("(j p) o -> p j o", p=P)
    bq4v = bq4_d.rearrange("(j p) o -> p j o", p=P)    # [128, 2, 1]
    bp0v = bp0_d.rearrange("(j p) o -> p j o", p=P)
    yv = y_d.rearrange("(j p) n -> p j n", p=P)        # [128, 2, 2048]

    with tile.TileContext(nc) as tc:
        with (
            tc.tile_pool(name="big", bufs=1) as big,
            tc.tile_pool(name="wts", bufs=1) as wts,
            tc.tile_pool(name="stats", bufs=1) as stats,
            tc.tile_pool(name="epool", bufs=6) as epool,
            tc.tile_pool(name="rp", bufs=2) as rp,
            tc.tile_pool(name="hap", bufs=2) as hap,
            tc.tile_pool(name="yp", bufs=2) as yp,
        ):
            dummy = wts.tile([P, 512], bf16)
            nc.vector.memset(dummy, 0.0)

            # ---- DMA: xb split sync/scalar, x8+consts on gpsimd,
            #      weights on sync after the xb chunks ----
            xs_b = big.tile([P, 2, N], bf16)
            xs_8 = big.tile([P, 2, N], f8)

            gm = wts.tile([P, 16], f32)
            nc.gpsimd.dma_start(out=gm, in_=gm_d)
            gt = wts.tile([16, P], f32)
            nc.gpsimd.dma_start(out=gt, in_=gt_d)
            nc.gpsimd.dma_start(out=xs_8, in_=x8v)
            ones8 = wts.tile([P, 2, P], f8)
            nc.gpsimd.memset(ones8, 1.0)

            st6 = stats.tile([P, 2, 8, 6], f32)
            for j in range(2):
                qeng = nc.sync if j == 0 else nc.scalar
                for qd in range(4):
                    sl = slice(qd * 1024, (qd + 1) * 1024)
                    qeng.dma_start(out=xs_b[:, j, sl], in_=xbv[:, j, sl])
                    for h in range(2):
                        sg = qd * 2 + h
                        ssl = slice(sg * 512, (sg + 1) * 512)
                        nc.vector.bn_stats(out=st6[:, j, sg, :],
                                           in_=xs_b[:, j, ssl])

            wqk_b = wts.tile([P, 2, 2 * C], bf16)
            nc.sync.dma_start(out=wqk_b, in_=wqkv)
            wv_b = wts.tile([P, 2, C], bf16)
            nc.sync.dma_start(out=wv_b, in_=wvv)
            wp_b = wts.tile([P, 2, C], bf16)
            nc.sync.dma_start(out=wp_b, in_=wpv)
            bq4 = wts.tile([P, 2, 1], f32)
            nc.sync.dma_start(out=bq4, in_=bq4v)
            bp0 = wts.tile([P, 2, 1], f32)
            nc.sync.dma_start(out=bp0, in_=bp0v)
            eps_t = wts.tile([16, 1], f32)
            nc.vector.memset(eps_t, EPS)

            # PE warmup while the x DMA lands
            with tc.tile_pool(name="psW", bufs=1, space="PSUM") as psw:
                wps = psw.tile([P, 512], f32, tag="w")
                for _ in range(14):
                    nc.tensor.matmul(wps, lhsT=dummy[:, 0:P], rhs=dummy,
                                     start=True, stop=True)

            # ---- group stats + runtime weight scaling, per j-half ----
            AB = stats.tile([P, 2, 2], f32)
            wqk_s = wts.tile([P, 2, 2 * C], f8)
            wv_s = wts.tile([P, 2, C], f8)
            mu8 = wts.tile([P, 2, 16], f8)
            with tc.tile_pool(name="psStat", bufs=1, space="PSUM") as psst:
                for j in range(2):
                    mv = stats.tile([P, 2], f32, tag="mv")
                    nc.vector.bn_aggr(out=mv, in_=st6[:, j])
                    t2 = stats.tile([P, 2], f32, tag="t2")
                    nc.vector.tensor_copy(out=t2[:, 0:1], in_=mv[:, 0:1])
                    nc.vector.scalar_tensor_tensor(
                        out=t2[:, 1:2], in0=mv[:, 0:1], scalar=mv[:, 0:1],
                        in1=mv[:, 1:2], op0=ALU.mult, op1=ALU.add,
                    )
                    gagg = psst.tile([16, 2], f32, tag="gagg")
                    nc.tensor.matmul(gagg, lhsT=gm, rhs=t2, start=True,
                                     stop=True)
                    grs = stats.tile([16, 2], f32, tag="grs")
                    nc.scalar.copy(out=grs[:, 0:1], in_=gagg[:, 0:1])
                    sq = stats.tile([16, 1], f32, tag="sq")
                    nc.scalar.square(out=sq, in_=gagg[:, 0:1])
                    var = stats.tile([16, 1], f32, tag="var")
                    nc.vector.tensor_sub(out=var, in0=gagg[:, 1:2], in1=sq)
                    nc.scalar.activation(out=var, in_=var, func=AF.Sqrt,
                                         bias=eps_t, scale=1.0)
                    nc.vector.reciprocal(out=grs[:, 1:2], in_=var)
                    gb = psst.tile([P, 2], f32, tag="gb")
                    nc.tensor.matmul(gb, lhsT=gt, rhs=grs, start=True,
                                     stop=True)
                    nc.scalar.copy(out=AB[:, j, :], in_=gb)
                    # k columns first: k-gen is the first consumer
                    nc.vector.tensor_scalar(
                        out=wqk_s[:, j, C:2 * C], in0=wqk_b[:, j, C:2 * C],
                        scalar1=AB[:, j, 1:2], scalar2=None, op0=ALU.mult)
                    nc.vector.tensor_scalar(
                        out=wqk_s[:, j, 0:C], in0=wqk_b[:, j, 0:C],
                        scalar1=AB[:, j, 1:2], scalar2=None, op0=ALU.mult)
                    nc.vector.tensor_scalar(
                        out=wv_s[:, j, :], in0=wv_b[:, j, :],
                        scalar1=AB[:, j, 1:2], scalar2=None, op0=ALU.mult)
                    nc.vector.tensor_scalar(
                        out=mu8[:, j, 0:1], in0=AB[:, j, 0:1],
                        scalar1=MUS, scalar2=None, op0=ALU.mult)
                # preload the exp table while ACT is otherwise idle
                dexp = stats.tile([16, 1], f32, tag="dexp")
                nc.scalar.activation(out=dexp, in_=eps_t, func=AF.Exp)

            # bias matvecs: bqe = bq4 - wqk_s[:, :, q]^T mu ;  cv = -wv_s^T mu
            cvs = wts.tile([P, 2, 16], bf16)
            bqe = wts.tile([P, 2, 1], f32)
            bpe = wts.tile([P, 2, 1], f32)
            with tc.tile_pool(name="psMv", bufs=2, space="PSUM") as psmv:
                for jo in range(2):
                    mq = psmv.tile([P, 1], f32, tag="mv0")
                    nc.tensor.matmul(mq, lhsT=wqk_s[:, :, jo * P:(jo + 1) * P],
                                     rhs=mu8[:, :, 0:1], start=True, stop=True,
                                     perf_mode=DR)
                    nc.vector.tensor_scalar(
                        out=bqe[:, jo, :], in0=mq, scalar1=-1.0 / MUS,
                        scalar2=bq4[:, jo, :], op0=ALU.mult, op1=ALU.add)
                    cm = psmv.tile([P, 1], f32, tag="mv1")
                    nc.tensor.matmul(cm, lhsT=wv_s[:, :, jo * P:(jo + 1) * P],
                                     rhs=mu8[:, :, 0:1], start=True, stop=True,
                                     perf_mode=DR)
                    nc.vector.tensor_scalar(
                        out=cvs[:, jo, 0:1], in0=cm, scalar1=-1.0 / MUS,
                        scalar2=None, op0=ALU.mult)

            # ---- main compute ----
            q_s = big.tile([P, 2, NH], f8)
            k_s = big.tile([P, 2, N], f8)
            v_s = big.tile([P, MC, C], f8)
            with (
                tc.tile_pool(name="psQK", bufs=2, space="PSUM") as psqk,
                tc.tile_pool(name="psAcc", bufs=1, space="PSUM") as psacc,
                tc.tile_pool(name="psPP", bufs=1, space="PSUM") as pspp,
            ):
                # bpe = bp0 + wp^T cv (pp-tag psum; needed only at stage2)
                for jo in range(2):
                    pb = pspp.tile([P, 512], f32, name=f"pb{jo}", tag="pp")
                    for j in range(2):
                        nc.tensor.matmul(
                            pb[:, 0:1], lhsT=wp_b[:, j, jo * P:(jo + 1) * P],
                            rhs=cvs[:, j, 0:1],
                            start=(j == 0), stop=(j == 1))
                    nc.vector.tensor_add(out=bpe[:, jo, :], in0=pb[:, 0:1],
                                         in1=bp0[:, jo, :])

                def kgen_full(tt):
                    sl = slice(tt * 512, (tt + 1) * 512)
                    kq = psqk.tile([P, 2, 512], f32, name=f"kq{tt}", tag="qk")
                    for jo in range(2):
                        nc.tensor.matmul(
                            kq[:, jo, :],
                            lhsT=wqk_s[:, :, C + jo * P:C + (jo + 1) * P],
                            rhs=xs_8[:, :, sl], start=True, stop=True,
                            perf_mode=DR)
                    nc.vector.tensor_copy(out=k_s[:, :, sl], in_=kq)

                def kgen_half(tt, h):
                    csl = slice(tt * 512 + h * 256, tt * 512 + (h + 1) * 256)
                    kt = pspp.tile([P, 512], f32, name=f"kt{tt}{h}", tag="pp")
                    for jo in range(2):
                        nc.tensor.matmul(
                            kt[:, jo * 256:(jo + 1) * 256],
                            lhsT=wqk_s[:, :, C + jo * P:C + (jo + 1) * P],
                            rhs=xs_8[:, :, csl], start=True, stop=True,
                            perf_mode=DR)
                    ktv = kt.rearrange("p (j c) -> p j c", c=256)
                    nc.vector.tensor_copy(out=k_s[:, :, csl], in_=ktv)

                def qgen_full(tt):
                    sl = slice(tt * 512, (tt + 1) * 512)
                    qp = psqk.tile([P, 2, 512], f32, name=f"qp{tt}", tag="qk")
                    for jo in range(2):
                        nc.tensor.matmul(
                            qp[:, jo, :],
                            lhsT=wqk_s[:, :, jo * P:(jo + 1) * P],
                            rhs=xs_8[:, :, sl], start=True, stop=True,
                            perf_mode=DR)
                    for jo in range(2):
                        nc.vector.tensor_scalar(
                            out=q_s[:, jo, sl], in0=qp[:, jo, :],
                            scalar1=bqe[:, jo, :], scalar2=None, op0=ALU.add)

                def qgen_half(tt, h):
                    csl = slice(tt * 512 + h * 256, tt * 512 + (h + 1) * 256)
                    qt = pspp.tile([P, 512], f32, name=f"qt{tt}{h}", tag="pp")
                    for jo in range(2):
                        nc.tensor.matmul(
                            qt[:, jo * 256:(jo + 1) * 256],
                            lhsT=wqk_s[:, :, jo * P:(jo + 1) * P],
                            rhs=xs_8[:, :, csl], start=True, stop=True,
                            perf_mode=DR)
                    for jo in range(2):
                        nc.vector.tensor_scalar(
                            out=q_s[:, jo, csl],
                            in0=qt[:, jo * 256:(jo + 1) * 256],
                            scalar1=bqe[:, jo, :], scalar2=None, op0=ALU.add)

                def vgen(pr):
                    vt = pspp.tile([P, 512], f32, name=f"vt{pr}", tag="pp")
                    vtv = vt.rearrange("p (i c) -> p i c", c=C)
                    for i in range(2):
                        mc = 2 * pr + i
                        nc.tensor.matmul(
                            vtv[:, i, :],
                            lhsT=xs_8[:, :, mc * P:(mc + 1) * P],
                            rhs=wv_s, start=True, stop=True, perf_mode=DR)
                    if pr in (3, 7, 11):
                        nc.scalar.copy(out=v_s[:, 2 * pr:2 * pr + 2, :],
                                       in_=vtv)
                    else:
                        nc.vector.tensor_copy(
                            out=v_s[:, 2 * pr:2 * pr + 2, :], in_=vtv)

                # prologue: k tt0/tt1 + q tt0 (qk tag), k tt2 halves (pp tag)
                kgen_full(0)
                kgen_full(1)
                kgen_half(2, 0)
                kgen_half(2, 1)
                qgen_full(0)

                def stage1(pend):
                    av0, av1, sp, psl, st = pend
                    rb = rp.tile([P, 512], f32, name="rb", tag="rb")
                    nc.vector.reciprocal(out=rb, in_=sp)
                    ha = hap.tile([P, 2, 512], bf16, name="ha", tag="ha")
                    nc.vector.tensor_mul(out=ha[:, 0, :], in0=av0, in1=rb)
                    nc.vector.tensor_mul(out=ha[:, 1, :], in0=av1, in1=rb)
                    st["ha"] = ha

                def stage2(pend):
                    psl = pend[3]
                    ha = pend[4]["ha"]
                    yt = yp.tile([P, 2, 512], bf16, name="yt", tag="yt")
                    for jo in range(2):
                        pp = pspp.tile([P, 512], f32, name="pp", tag="pp")
                        for j in range(2):
                            nc.tensor.matmul(
                                pp, lhsT=wp_b[:, j, jo * P:(jo + 1) * P],
                                rhs=ha[:, j, :],
                                start=(j == 0), stop=(j == 1))
                        nc.vector.scalar_tensor_tensor(
                            out=yt[:, jo, :], in0=pp, scalar=bpe[:, jo, :],
                            in1=xs_b[:, jo, psl], op0=ALU.add, op1=ALU.add)
                    nc.sync.dma_start(out=yv[:, :, psl], in_=yt)

                pend = None
                for tt in range(NT):
                    sl = slice(tt * 512, (tt + 1) * 512)
                    av0 = psacc.tile([P, 512], f32, name="av0", tag="av0")
                    av1 = psacc.tile([P, 512], f32, name="av1", tag="av1")
                    sp = psacc.tile([P, 512], f32, name="sp", tag="sp")
                    ets = [None] * MCP

                    def avs(pr, av0=av0, av1=av1, sp=sp, ets=ets):
                        et = ets[pr]
                        vsl = v_s[:, 2 * pr:2 * pr + 2, :]
                        first, last = pr == 0, pr == MCP - 1
                        nc.tensor.matmul(av0, lhsT=vsl[:, :, 0:P], rhs=et,
                                         start=first, stop=last, perf_mode=DR)
                        nc.tensor.matmul(av1, lhsT=vsl[:, :, P:C], rhs=et,
                                         start=first, stop=last, perf_mode=DR)
                        nc.tensor.matmul(sp, lhsT=ones8, rhs=et,
                                         start=first, stop=last, perf_mode=DR)

                    for pr in range(MCP):
                        if pend is not None:
                            if pr == 0:
                                stage1(pend)
                            elif pr == 5:
                                stage2(pend)
                                pend = None
                        qkp = psqk.tile([P, 2, 512], f32, name="qkp",
                                        tag="qk")
                        for i in range(2):
                            mc = 2 * pr + i
                            nc.tensor.matmul(
                                qkp[:, i, :],
                                lhsT=k_s[:, :, mc * P:(mc + 1) * P],
                                rhs=q_s[:, :, sl], start=True, stop=True,
                                perf_mode=DR)
                        et = epool.tile([P, 2, 512], f8, name=f"et{pr % 6}",
                                        tag="et")
                        nc.scalar.activation(out=et, in_=qkp, func=AF.Exp,
                                             scale=1.0 / (16.0 * SQ * SQ))
                        ets[pr] = et
                        if tt == 0:
                            vgen(pr)
                            if pr < 10:
                                kgen_half(3 + pr // 2, pr % 2)
                            elif pr < 12:
                                qgen_half(1, pr - 10)
                        elif tt < 3 and pr < 2:
                            qgen_half(tt + 1, pr)
                        if pr >= LAG:
                            avs(pr - LAG)
                    for pr in range(MCP - LAG, MCP):
                        avs(pr)
                    pend = (av0, av1, sp, sl, {})
                stage1(pend)
                stage2(pend)

    nc.compile()
    return nc


def _get_prog():
    global _prog
    if _prog is None:
        _prog = _build_program()
    return _prog


def _host_prep(x, gn_w, gn_b, qkv_w, qkv_b, proj_w, proj_b):
    """Returns (shared input dict, per-core xb list, per-core x8 list)."""
    import ml_dtypes

    x = np.asarray(x, dtype=np.float32)
    gn_w = np.asarray(gn_w, dtype=np.float32)
    gn_b = np.asarray(gn_b, dtype=np.float32)
    qkv_w = np.asarray(qkv_w, dtype=np.float32)
    qkv_b = np.asarray(qkv_b, dtype=np.float32)
    proj_w = np.asarray(proj_w, dtype=np.float32)
    proj_b = np.asarray(proj_b, dtype=np.float32)

    bf = ml_dtypes.bfloat16
    f8 = ml_dtypes.float8_e4m3

    Wq = qkv_w[0:C] * gn_w[None, :]
    Wk = qkv_w[C:2 * C] * gn_w[None, :]
    Wv = qkv_w[2 * C:3 * C] * gn_w[None, :]
    bq_h = qkv_w[0:C] @ gn_b + qkv_b[0:C]
    bv_h = qkv_w[2 * C:3 * C] @ gn_b + qkv_b[2 * C:3 * C]
    bp_h = proj_b + proj_w @ bv_h

    wqk = (SQ * np.concatenate([Wq.T, Wk.T], axis=1)).astype(bf)  # [C, 2C]
    wv_h = np.ascontiguousarray(Wv.T).astype(bf)
    wp_h = np.ascontiguousarray(proj_w.T).astype(bf)

    cidx = np.arange(P)
    gm = np.zeros((P, 16), dtype=np.float32)
    gm[cidx, cidx // GSIZE] = 1.0 / GSIZE
    gt = np.zeros((16, P), dtype=np.float32)
    gt[cidx // GSIZE, cidx] = 1.0

    shared = {
        "wqk": wqk,
        "wv": wv_h,
        "wp": wp_h,
        "bq4": (SQ * bq_h).reshape(C, 1).astype(np.float32),
        "bp0": bp_h.reshape(C, 1).astype(np.float32),
        "gm": gm,
        "gt": gt,
    }

    xf = x.reshape(B, C, N)
    xb_per_core = []
    x8_per_core = []
    for core in range(NCORES):
        b, half = core // 2, core % 2
        if half == 0:
            xc = xf[b]
        else:
            xc = np.concatenate([xf[b][:, NH:], xf[b][:, :NH]], axis=1)
        xb_per_core.append(np.ascontiguousarray(xc).astype(bf))
        x8_per_core.append(
            np.clip(np.ascontiguousarray(xc), -240, 240).astype(f8))
    return shared, xb_per_core, x8_per_core


def run_sharded(inputs, trace=False, trace_kwargs=None):
    """Run the 8-core kernel. Returns (full_output, BassKernelResults)."""
    from concourse.bass_utils import run_bass_kernel_spmd

    nc = _get_prog()
    shared, xb_per_core, x8_per_core = _host_prep(**inputs)
    in_maps = [{**shared, "xb": xb_per_core[c], "x8": x8_per_core[c]}
               for c in range(NCORES)]
    kw = {}
    if trace:
        kw["trace"] = True
        if trace_kwargs:
            kw["trace_kwargs"] = trace_kwargs
    res = run_bass_kernel_spmd(nc, in_maps, list(range(NCORES)), **kw)

    out = np.empty((B, C, N), dtype=np.float32)
    for core in range(NCORES):
        b, half = core // 2, core % 2
        yc = np.asarray(res.results[core]["y"], dtype=np.float32)
        out[b][:, half * NH:(half + 1) * NH] = yc
    return out.reshape(B, C, HH, WW), res


def kernel(**inputs):
    out, _ = run_sharded(inputs)
    return out


# revision 8
# speedup vs baseline: 1.1855x; 1.1855x over previous
"""Trainium2 Bass kernel for nn_AttentionBlock (GroupNorm -> 1x1 qkv conv ->
softmax attention over N=HW -> 1x1 proj -> residual).

Sharding: 8 cores = 4 images x 2 query-column halves. Each core receives its
image column-permuted so its own 2048 query columns come first; attention is
permutation-invariant over key/value positions, so k/v use all 4096 columns
in permuted order.

v3 design (fp8 DoubleRow attention, pipelined generation):
  - x uploaded twice: bf16 (stats + residual) and fp8e4m3 (matmul operand).
  - GroupNorm folded into qkv weights at runtime (wqk_s = wqk * r_c); q/k
    carry a x4 fp8 scale compensated in the exp scale 1/256. The -mu*r
    correction folds into the q bias (matvec) and proj bias (matvec chain);
    for k it is softmax-invariant and dropped.
  - q/k/v fp8 in [128, 2, n] channel-pair layout; qk/av/S matmuls run fp8
    DoubleRow (contraction 256/pass). S accumulates on the PE via an
    all-ones fp8 matmul into a replicated [128,512] PSUM bank.
  - exp on ACT per PAIR of key chunks ([128,2,512] PSUM read); ACT does
    nothing else in steady state.
  - k/q/v generation is interleaved INTO attention tile 0's pair loop
    (v-gen on the qk PSUM tag, k/q halves on the pp tag) so the exp stream
    starts ~20us in and generation copies hide under it.
  - av/S accumulation lags exp by 4 pairs so the tile-boundary recip+mul
    (DVE) never stalls the PE's write-after-read on the accumulators.
  - proj stays bf16; y stored bf16, upcast on host.
"""

import numpy as np

B, C, HH, WW = 4, 256, 64, 64
N = HH * WW            # 4096
NH = N // 2            # 2048 query columns per core
GROUPS = 32
GSIZE = C // GROUPS    # 8
EPS = 1e-5
NCORES = 8
P = 128
NT = NH // 512         # 4 query tiles per core
MC = N // P            # 32 key chunks
MCP = MC // 2          # 16 key-chunk pairs
KT = N // 512          # 8 column tiles for k
SQ = 4.0               # q,k fp8 pre-scale (folded into wqk host upload)
MUS = 32.0             # mu fp8 pre-scale (compensated in bias fixups)
LAG = 13               # av/S accumulation lag behind exp, in pairs

_prog = None


def _build_program():
    import concourse.bacc as bacc
    import concourse.tile as tile
    from concourse import mybir

    f32 = mybir.dt.float32
    bf16 = mybir.dt.bfloat16
    f8 = mybir.dt.float8e4
    AF = mybir.ActivationFunctionType
    ALU = mybir.AluOpType
    DR = mybir.MatmulPerfMode.DoubleRow

    nc = bacc.Bacc("TRN2", target_bir_lowering=False, debug=False,
                   num_devices=NCORES)

    xb_d = nc.dram_tensor("xb", [C, N], bf16, kind="ExternalInput").ap()
    x8_d = nc.dram_tensor("x8", [C, N], f8, kind="ExternalInput").ap()
    wqk_d = nc.dram_tensor("wqk", [C, 2 * C], bf16, kind="ExternalInput").ap()
    wv_d = nc.dram_tensor("wv", [C, C], bf16, kind="ExternalInput").ap()
    wp_d = nc.dram_tensor("wp", [C, C], bf16, kind="ExternalInput").ap()
    bq4_d = nc.dram_tensor("bq4", [C, 1], f32, kind="ExternalInput").ap()
    bp0_d = nc.dram_tensor("bp0", [C, 1], f32, kind="ExternalInput").ap()
    gm_d = nc.dram_tensor("gm", [P, 16], f32, kind="ExternalInput").ap()
    gt_d = nc.dram_tensor("gt", [16, P], f32, kind="ExternalInput").ap()
    y_d = nc.dram_tensor("y", [C, NH], bf16, kind="ExternalOutput").ap()

    xbv = xb_d.rearrange("(j p) n -> p j n", p=P)      # [128, 2, 4096]
    x8v = x8_d.rearrange("(j p) n -> p j n", p=P)
    wqkv = wqk_d.rearrange("(j p) o -> p j o", p=P)    # [128, 2, 512]
    wvv = wv_d.rearrange("(j p) o -> p j o", p=P)
    wpv = wp_d.rearrange("(j p) o -> p j o", p=P)
    bq4v = bq4_d.rearrange("(j p) o -> p j o", p=P)    # [128, 2, 1]
    bp0v = bp0_d.rearrange("(j p) o -> p j o", p=P)
    yv = y_d.rearrange("(j p) n -> p j n", p=P)        # [128, 2, 2048]

    with tile.TileContext(nc) as tc:
        with (
            tc.tile_pool(name="big", bufs=1) as big,
            tc.tile_pool(name="wts", bufs=1) as wts,
            tc.tile_pool(name="stats", bufs=1) as stats,
            tc.tile_pool(name="epool", bufs=16) as epool,
            tc.tile_pool(name="rp", bufs=2) as rp,
            tc.tile_pool(name="hap", bufs=2) as hap,
            tc.tile_pool(name="yp", bufs=2) as yp,
        ):
            dummy = wts.tile([P, 512], bf16)
            nc.vector.memset(dummy, 0.0)

            # ---- DMA: xb split sync/scalar, x8+consts on gpsimd,
            #      weights on sync after the xb chunks ----
            xs_b = big.tile([P, 2, N], bf16)
            xs_8 = big.tile([P, 2, N], f8)

            gm = wts.tile([P, 16], f32)
            nc.gpsimd.dma_start(out=gm, in_=gm_d)
            gt = wts.tile([16, P], f32)
            nc.gpsimd.dma_start(out=gt, in_=gt_d)
            nc.gpsimd.dma_start(out=xs_8, in_=x8v)
            ones8 = wts.tile([P, 2, P], f8)
            nc.gpsimd.memset(ones8, 1.0)

            st6 = stats.tile([P, 2, 8, 6], f32)
            for j in range(2):
                qeng = nc.sync if j == 0 else nc.scalar
                for qd in range(4):
                    sl = slice(qd * 1024, (qd + 1) * 1024)
                    qeng.dma_start(out=xs_b[:, j, sl], in_=xbv[:, j, sl])
                    for h in range(2):
                        sg = qd * 2 + h
                        ssl = slice(sg * 512, (sg + 1) * 512)
                        nc.vector.bn_stats(out=st6[:, j, sg, :],
                                           in_=xs_b[:, j, ssl])

            wqk_b = wts.tile([P, 2, 2 * C], bf16)
            nc.sync.dma_start(out=wqk_b, in_=wqkv)
            wv_b = wts.tile([P, 2, C], bf16)
            nc.sync.dma_start(out=wv_b, in_=wvv)
            wp_b = wts.tile([P, 2, C], bf16)
            nc.sync.dma_start(out=wp_b, in_=wpv)
            bq4 = wts.tile([P, 2, 1], f32)
            nc.sync.dma_start(out=bq4, in_=bq4v)
            bp0 = wts.tile([P, 2, 1], f32)
            nc.sync.dma_start(out=bp0, in_=bp0v)
            eps_t = wts.tile([16, 1], f32)
            nc.vector.memset(eps_t, EPS)

            # PE warmup while the x DMA lands
            with tc.tile_pool(name="psW", bufs=1, space="PSUM") as psw:
                wps = psw.tile([P, 512], f32, tag="w")
                for _ in range(8):
                    nc.tensor.matmul(wps, lhsT=dummy[:, 0:P], rhs=dummy,
                                     start=True, stop=True)

            # ---- group stats + runtime weight scaling, per j-half ----
            AB = stats.tile([P, 2, 2], f32)
            wqk_s = wts.tile([P, 2, 2 * C], f8)
            wv_s = wts.tile([P, 2, C], f8)
            mu8 = wts.tile([P, 2, 16], f8)
            with tc.tile_pool(name="psStat", bufs=1, space="PSUM") as psst:
                mvs, t2s, gaggs, grss, gbs = [], [], [], [], []
                for j in range(2):
                    mv = stats.tile([P, 2], f32, tag=f"mv{j}", name=f"mv{j}")
                    nc.vector.bn_aggr(out=mv, in_=st6[:, j])
                    mvs.append(mv)
                for j in range(2):
                    t2 = stats.tile([P, 2], f32, tag=f"t2{j}", name=f"t2{j}")
                    nc.vector.tensor_copy(out=t2[:, 0:1], in_=mvs[j][:, 0:1])
                    nc.vector.scalar_tensor_tensor(
                        out=t2[:, 1:2], in0=mvs[j][:, 0:1],
                        scalar=mvs[j][:, 0:1], in1=mvs[j][:, 1:2],
                        op0=ALU.mult, op1=ALU.add)
                    t2s.append(t2)
                for j in range(2):
                    gagg = psst.tile([16, 2], f32, tag=f"gagg{j}",
                                     name=f"gagg{j}")
                    nc.tensor.matmul(gagg, lhsT=gm, rhs=t2s[j], start=True,
                                     stop=True)
                    gaggs.append(gagg)
                for j in range(2):
                    grs = stats.tile([16, 2], f32, tag=f"grs{j}",
                                     name=f"grs{j}")
                    nc.scalar.copy(out=grs[:, 0:1], in_=gaggs[j][:, 0:1])
                    sq = stats.tile([16, 1], f32, tag=f"sq{j}", name=f"sq{j}")
                    nc.scalar.square(out=sq, in_=gaggs[j][:, 0:1])
                    var = stats.tile([16, 1], f32, tag=f"var{j}",
                                     name=f"var{j}")
                    nc.vector.tensor_sub(out=var, in0=gaggs[j][:, 1:2],
                                         in1=sq)
                    nc.scalar.activation(out=var, in_=var, func=AF.Sqrt,
                                         bias=eps_t, scale=1.0)
                    nc.vector.reciprocal(out=grs[:, 1:2], in_=var)
                    grss.append(grs)
                for j in range(2):
                    gb = psst.tile([P, 2], f32, tag=f"gb{j}", name=f"gb{j}")
                    nc.tensor.matmul(gb, lhsT=gt, rhs=grss[j], start=True,
                                     stop=True)
                    gbs.append(gb)
                for j in range(2):
                    nc.scalar.copy(out=AB[:, j, :], in_=gbs[j])
                for j in range(2):
                    nc.vector.tensor_scalar(
                        out=wqk_s[:, j, C:2 * C], in0=wqk_b[:, j, C:2 * C],
                        scalar1=AB[:, j, 1:2], scalar2=None, op0=ALU.mult)
                for j in range(2):
                    nc.vector.tensor_scalar(
                        out=wqk_s[:, j, 0:C], in0=wqk_b[:, j, 0:C],
                        scalar1=AB[:, j, 1:2], scalar2=None, op0=ALU.mult)
                    nc.vector.tensor_scalar(
                        out=wv_s[:, j, :], in0=wv_b[:, j, :],
                        scalar1=AB[:, j, 1:2], scalar2=None, op0=ALU.mult)
                    nc.vector.tensor_scalar(
                        out=mu8[:, j, 0:1], in0=AB[:, j, 0:1],
                        scalar1=MUS, scalar2=None, op0=ALU.mult)
                # preload the exp table while ACT is otherwise idle
                dexp = stats.tile([16, 1], f32, tag="dexp")
                nc.scalar.activation(out=dexp, in_=eps_t, func=AF.Exp)

            # bias matvecs: bqe = bq4 - wqk_s[:, :, q]^T mu ;  cv = -wv_s^T mu
            cvs = wts.tile([P, 2, 16], bf16)
            bqe = wts.tile([P, 2, 1], f32)
            bpe = wts.tile([P, 2, 1], f32)
            with tc.tile_pool(name="psMv", bufs=2, space="PSUM") as psmv:
                for jo in range(2):
                    mq = psmv.tile([P, 1], f32, tag="mv0")
                    nc.tensor.matmul(mq, lhsT=wqk_s[:, :, jo * P:(jo + 1) * P],
                                     rhs=mu8[:, :, 0:1], start=True, stop=True,
                                     perf_mode=DR)
                    nc.vector.tensor_scalar(
                        out=bqe[:, jo, :], in0=mq, scalar1=-1.0 / MUS,
                        scalar2=bq4[:, jo, :], op0=ALU.mult, op1=ALU.add)
                    cm = psmv.tile([P, 1], f32, tag="mv1")
                    nc.tensor.matmul(cm, lhsT=wv_s[:, :, jo * P:(jo + 1) * P],
                                     rhs=mu8[:, :, 0:1], start=True, stop=True,
                                     perf_mode=DR)
                    nc.vector.tensor_scalar(
                        out=cvs[:, jo, 0:1], in0=cm, scalar1=-1.0 / MUS,
                        scalar2=None, op0=ALU.mult)

            # ---- main compute ----
            q_s = big.tile([P, 2, NH], f8)
            k_s = big.tile([P, 2, N], f8)
            v_s = big.tile([P, MC, C], f8)
            with (
                tc.tile_pool(name="psQK", bufs=2, space="PSUM") as psqk,
                tc.tile_pool(name="psAcc", bufs=1, space="PSUM") as psacc,
                tc.tile_pool(name="psPP", bufs=1, space="PSUM") as pspp,
            ):
                # bpe = bp0 + wp^T cv (pp-tag psum; needed only at stage2)
                for jo in range(2):
                    pb = pspp.tile([P, 512], f32, name=f"pb{jo}", tag="pp")
                    for j in range(2):
                        nc.tensor.matmul(
                            pb[:, 0:1], lhsT=wp_b[:, j, jo * P:(jo + 1) * P],
                            rhs=cvs[:, j, 0:1],
                            start=(j == 0), stop=(j == 1))
                    nc.vector.tensor_add(out=bpe[:, jo, :], in0=pb[:, 0:1],
                                         in1=bp0[:, jo, :])

                def kgen_full(tt):
                    sl = slice(tt * 512, (tt + 1) * 512)
                    kq = psqk.tile([P, 2, 512], f32, name=f"kq{tt}", tag="qk")
                    for jo in range(2):
                        nc.tensor.matmul(
                            kq[:, jo, :],
                            lhsT=wqk_s[:, :, C + jo * P:C + (jo + 1) * P],
                            rhs=xs_8[:, :, sl], start=True, stop=True,
                            perf_mode=DR)
                    nc.vector.tensor_copy(out=k_s[:, :, sl], in_=kq)

                def kgen_half(tt, h):
                    csl = slice(tt * 512 + h * 256, tt * 512 + (h + 1) * 256)
                    kt = pspp.tile([P, 512], f32, name=f"kt{tt}{h}", tag="pp")
                    for jo in range(2):
                        nc.tensor.matmul(
                            kt[:, jo * 256:(jo + 1) * 256],
                            lhsT=wqk_s[:, :, C + jo * P:C + (jo + 1) * P],
                            rhs=xs_8[:, :, csl], start=True, stop=True,
                            perf_mode=DR)
                    ktv = kt.rearrange("p (j c) -> p j c", c=256)
                    nc.vector.tensor_copy(out=k_s[:, :, csl], in_=ktv)

                def qgen_full(tt):
                    sl = slice(tt * 512, (tt + 1) * 512)
                    qp = psqk.tile([P, 2, 512], f32, name=f"qp{tt}", tag="qk")
                    for jo in range(2):
                        nc.tensor.matmul(
                            qp[:, jo, :],
                            lhsT=wqk_s[:, :, jo * P:(jo + 1) * P],
                            rhs=xs_8[:, :, sl], start=True, stop=True,
                            perf_mode=DR)
                    for jo in range(2):
                        nc.vector.tensor_scalar(
                            out=q_s[:, jo, sl], in0=qp[:, jo, :],
                            scalar1=bqe[:, jo, :], scalar2=None, op0=ALU.add)

                def qgen_half(tt, h):
                    csl = slice(tt * 512 + h * 256, tt * 512 + (h + 1) * 256)
                    qt = pspp.tile([P, 512], f32, name=f"qt{tt}{h}", tag="pp")
                    for jo in range(2):
                        nc.tensor.matmul(
                            qt[:, jo * 256:(jo + 1) * 256],
                            lhsT=wqk_s[:, :, jo * P:(jo + 1) * P],
                            rhs=xs_8[:, :, csl], start=True, stop=True,
                            perf_mode=DR)
                    for jo in range(2):
                        nc.vector.tensor_scalar(
                            out=q_s[:, jo, csl],
                            in0=qt[:, jo * 256:(jo + 1) * 256],
                            scalar1=bqe[:, jo, :], scalar2=None, op0=ALU.add)

                def vgen(vidx):
                    vt = pspp.tile([P, 512], f32, name=f"vt{vidx}", tag="pp")
                    vtv = vt.rearrange("p (i c) -> p i c", c=C)
                    for i in range(2):
                        mc = 2 * vidx + i
                        nc.tensor.matmul(
                            vtv[:, i, :],
                            lhsT=xs_8[:, :, mc * P:(mc + 1) * P],
                            rhs=wv_s, start=True, stop=True, perf_mode=DR)
                    if vidx % 4 == 3:
                        nc.scalar.copy(out=v_s[:, 2 * vidx:2 * vidx + 2, :],
                                       in_=vtv)
                    else:
                        nc.vector.tensor_copy(
                            out=v_s[:, 2 * vidx:2 * vidx + 2, :], in_=vtv)

                # prologue: k tt0/tt1 + q tt0 (qk tag), k tt2 halves (pp tag)
                kgen_full(0)
                kgen_full(1)
                kgen_half(2, 0)
                kgen_half(2, 1)
                qgen_full(0)

                def stage1(pend):
                    av0, av1, sp, st = pend[0], pend[1], pend[2], pend[5]
                    rb = rp.tile([P, 512], f32, name="rb", tag="rb")
                    nc.vector.reciprocal(out=rb, in_=sp)
                    ha = hap.tile([P, 2, 512], bf16, name="ha", tag="ha")
                    nc.vector.tensor_mul(out=ha[:, 0, :], in0=av0, in1=rb)
                    nc.vector.tensor_mul(out=ha[:, 1, :], in0=av1, in1=rb)
                    st["ha"] = ha

                def stage2(pend):
                    psl = pend[3]
                    ha = pend[5]["ha"]
                    yt = yp.tile([P, 2, 512], bf16, name="yt", tag="yt")
                    for jo in range(2):
                        pp = pspp.tile([P, 512], f32, name="pp", tag="pp")
                        for j in range(2):
                            nc.tensor.matmul(
                                pp, lhsT=wp_b[:, j, jo * P:(jo + 1) * P],
                                rhs=ha[:, j, :],
                                start=(j == 0), stop=(j == 1))
                        nc.vector.scalar_tensor_tensor(
                            out=yt[:, jo, :], in0=pp, scalar=bpe[:, jo, :],
                            in1=xs_b[:, jo, psl], op0=ALU.add, op1=ALU.add)
                    nc.sync.dma_start(out=yv[:, :, psl], in_=yt)

                # per-pr drain count for the 13 deferred old-tile avs pairs
                DRAIN_N = [1, 1, 1, 1, 1, 1, 2, 2, 2, 1, 0, 0, 0, 0, 0, 0]
                pend = None
                for tt in range(NT):
                    sl = slice(tt * 512, (tt + 1) * 512)
                    av0 = psacc.tile([P, 512], f32, name="av0", tag="av0")
                    av1 = psacc.tile([P, 512], f32, name="av1", tag="av1")
                    sp = psacc.tile([P, 512], f32, name="sp", tag="sp")
                    ets = [None] * MCP

                    def avs(pr, av0=av0, av1=av1, sp=sp, ets=ets):
                        et = ets[pr]
                        vsl = v_s[:, 2 * pr:2 * pr + 2, :]
                        first, last = pr == 0, pr == MCP - 1
                        nc.tensor.matmul(av0, lhsT=vsl[:, :, 0:P], rhs=et,
                                         start=first, stop=last, perf_mode=DR)
                        nc.tensor.matmul(av1, lhsT=vsl[:, :, P:C], rhs=et,
                                         start=first, stop=last, perf_mode=DR)
                        nc.tensor.matmul(sp, lhsT=ones8, rhs=et,
                                         start=first, stop=last, perf_mode=DR)

                    drain = list(range(LAG - 10, MCP)) if pend else []
                    for pr in range(MCP):
                        qkp = psqk.tile([P, 2, 512], f32, name="qkp",
                                        tag="qk")
                        for i in range(2):
                            mc = 2 * pr + i
                            nc.tensor.matmul(
                                qkp[:, i, :],
                                lhsT=k_s[:, :, mc * P:(mc + 1) * P],
                                rhs=q_s[:, :, sl], start=True, stop=True,
                                perf_mode=DR)
                        et = epool.tile([P, 2, 512], f8, name="et", tag="et")
                        nc.scalar.activation(out=et, in_=qkp, func=AF.Exp,
                                             scale=1.0 / (16.0 * SQ * SQ))
                        ets[pr] = et
                        if pend is not None:
                            for _ in range(DRAIN_N[pr]):
                                if drain:
                                    pend[4](drain.pop(0))
                            if pr == 10:
                                stage1(pend)
                            elif pr == 14:
                                stage2(pend)
                                pend = None
                        if tt == 0:
                            if pr < 8:
                                vgen(pr)
                            if pr < 10:
                                kgen_half(3 + pr // 2, pr % 2)
                            elif pr < 12:
                                qgen_half(1, pr - 10)
                        elif tt == 1:
                            if pr < 8:
                                vgen(8 + pr)
                            elif 10 <= pr < 12:
                                qgen_half(2, pr - 10)
                        elif tt == 2:
                            if 10 <= pr < 12:
                                qgen_half(3, pr - 10)
                        if pr >= LAG:
                            avs(pr - LAG)
                    pend = (av0, av1, sp, sl, avs, {})
                for k in range(LAG - 10, MCP):
                    pend[4](k)
                stage1(pend)
                stage2(pend)

    nc.compile()
    return nc


def _get_prog():
    global _prog
    if _prog is None:
        _prog = _build_program()
    return _prog


def _host_prep(x, gn_w, gn_b, qkv_w, qkv_b, proj_w, proj_b):
    """Returns (shared input dict, per-core xb list, per-core x8 list)."""
    import ml_dtypes

    x = np.asarray(x, dtype=np.float32)
    gn_w = np.asarray(gn_w, dtype=np.float32)
    gn_b = np.asarray(gn_b, dtype=np.float32)
    qkv_w = np.asarray(qkv_w, dtype=np.float32)
    qkv_b = np.asarray(qkv_b, dtype=np.float32)
    proj_w = np.asarray(proj_w, dtype=np.float32)
    proj_b = np.asarray(proj_b, dtype=np.float32)

    bf = ml_dtypes.bfloat16
    f8 = ml_dtypes.float8_e4m3

    Wq = qkv_w[0:C] * gn_w[None, :]
    Wk = qkv_w[C:2 * C] * gn_w[None, :]
    Wv = qkv_w[2 * C:3 * C] * gn_w[None, :]
    bq_h = qkv_w[0:C] @ gn_b + qkv_b[0:C]
    bv_h = qkv_w[2 * C:3 * C] @ gn_b + qkv_b[2 * C:3 * C]
    bp_h = proj_b + proj_w @ bv_h

    wqk = (SQ * np.concatenate([Wq.T, Wk.T], axis=1)).astype(bf)  # [C, 2C]
    wv_h = np.ascontiguousarray(Wv.T).astype(bf)
    wp_h = np.ascontiguousarray(proj_w.T).astype(bf)

    cidx = np.arange(P)
    gm = np.zeros((P, 16), dtype=np.float32)
    gm[cidx, cidx // GSIZE] = 1.0 / GSIZE
    gt = np.zeros((16, P), dtype=np.float32)
    gt[cidx // GSIZE, cidx] = 1.0

    shared = {
        "wqk": wqk,
        "wv": wv_h,
        "wp": wp_h,
        "bq4": (SQ * bq_h).reshape(C, 1).astype(np.float32),
        "bp0": bp_h.reshape(C, 1).astype(np.float32),
        "gm": gm,
        "gt": gt,
    }

    xf = x.reshape(B, C, N)
    xb_per_core = []
    x8_per_core = []
    for core in range(NCORES):
        b, half = core // 2, core % 2
        if half == 0:
            xc = xf[b]
        else:
            xc = np.concatenate([xf[b][:, NH:], xf[b][:, :NH]], axis=1)
        xb_per_core.append(np.ascontiguousarray(xc).astype(bf))
        x8_per_core.append(
            np.clip(np.ascontiguousarray(xc), -240, 240).astype(f8))
    return shared, xb_per_core, x8_per_core


def run_sharded(inputs, trace=False, trace_kwargs=None):
    """Run the 8-core kernel. Returns (full_output, BassKernelResults)."""
    from concourse.bass_utils import run_bass_kernel_spmd

    nc = _get_prog()
    shared, xb_per_core, x8_per_core = _host_prep(**inputs)
    in_maps = [{**shared, "xb": xb_per_core[c], "x8": x8_per_core[c]}
               for c in range(NCORES)]
    kw = {}
    if trace:
        kw["trace"] = True
        if trace_kwargs:
            kw["trace_kwargs"] = trace_kwargs
    res = run_bass_kernel_spmd(nc, in_maps, list(range(NCORES)), **kw)

    out = np.empty((B, C, N), dtype=np.float32)
    for core in range(NCORES):
        b, half = core // 2, core % 2
        yc = np.asarray(res.results[core]["y"], dtype=np.float32)
        out[b][:, half * NH:(half + 1) * NH] = yc
    return out.reshape(B, C, HH, WW), res


def kernel(**inputs):
    out, _ = run_sharded(inputs)
    return out
